# revision 28
# baseline (speedup 1.0000x reference)
"""AdaFace loss kernel for 8 TRN2 NeuronCores (raw Bass, hand-scheduled).

Sharding: class dimension (C=100000) split across 8 cores -> [1024, 12500]
f32 shard per core (partial-FC / vocab parallel); labels/norms replicated.

Math: for logits x in (-0.99, 0.99), arccos(x) lies strictly inside
[eps, pi-eps], so cos(clip(arccos(x), eps, pi-eps)) == x for every column
except the (row, label) entry of positive rows.  Hence

    out = 64 * x                 everywhere, plus
    out[r, l_r] = 64 * (cos(clip(arccos(x_rl) + g_ang_r, eps, pi-eps)) - g_add_r)

The per-row correction is injected *in the stream* as an additive delta
tile built with one fused tensor_scalar op:  d = (ramp == l_r) * delta_r,
y = 64*x + d.  delta_r = 64*(v_r - x_rl); the AdaFace margin statistics
(mean/unbiased-std of clipped feature norms over positive rows) are
computed on device with DVE free-dim reductions + a PE ones-matmul for the
partition-dim reduce-and-broadcast.

cos(theta+g) is evaluated without arccos via the identity
    cos(arccos(x)+g) = x*cos(g) - sqrt(1-x^2)*sin(g)
and the theta-space clip maps to x-space threshold tests:
    theta+g < eps      <=>  (g <= eps)  and  x > cos(eps-g)
    theta+g > pi-eps   <=>  (g >= -eps) and  x < -cos(eps+g)

All DMAs ride the single gpsimd SWDGE queue (FIFO); the stream is a
hand-rolled double-buffered pipeline with 4 semaphores, so every
instruction carries at most ONE sync wait (this walrus build rejects
more).
"""

import math
import sys
from contextlib import ExitStack

import numpy as np

sys.path.insert(0, "/opt/trn_rl_repo")

# ---- problem constants (hardcoded per instructions) ----
B = 1024
C = 100000
NCORES = 8
CSH = C // NCORES          # 12500 columns per core
NSH = B * CSH              # flat shard length
P = 128                    # partitions
RB = B // P                # 8 row blocks
T = 6250                   # free-dim tile (even -> DVE 2x fp32 tensor_scalar)
CT = CSH // T              # 2 column tiles per row block
NTILES = RB * CT           # 16 stream tiles
M_C = 0.4
EPS = 1e-3
S = 64.0
COS_EPS = math.cos(EPS)
PI = math.pi

_CACHED = {}


def _build_program():
    import concourse.bass as bass
    from concourse import mybir

    f32 = mybir.dt.float32
    u32 = mybir.dt.uint32
    Alu = mybir.AluOpType
    Act = mybir.ActivationFunctionType
    AxX = mybir.AxisListType.X

    nc = bass.Bass()

    lg = nc.declare_dram_parameter("logits", [NSH], f32, isOutput=False)
    # packed sidecar: [0:8]=norms [8:16]=posf [16:24]=mmask [24:32]=locf
    # [32:40]=xv (logits at label columns, replicated)
    sdc = nc.declare_dram_parameter("sidecar", [P, 5 * RB], f32, isOutput=False)
    rmp = nc.declare_dram_parameter("ramp", [P, T], f32, isOutput=False)
    out = nc.declare_dram_parameter("out", [NSH], f32, isOutput=True)

    lg2d = lg[:].rearrange("(a b) -> a b", b=CSH)
    out2d = out[:].rearrange("(a b) -> a b", b=CSH)

    def tileslice(dram2d, k):
        rb, ct = divmod(k, CT)
        return dram2d[rb * P : (rb + 1) * P, ct * T : (ct + 1) * T]

    ctx = ExitStack()

    def sb(name, shape, dtype=f32):
        return ctx.enter_context(nc.sbuf_tensor(name, shape, dtype))[:]

    def psb(name, shape):
        return ctx.enter_context(nc.psum_tensor(name, shape, f32))[:]

    def sem(name):
        return ctx.enter_context(nc.semaphore(name))

    with ctx:
        sd = sb("sd", [P, 5 * RB])
        ramp = sb("ramp_t", [P, T])
        xt = [sb("x0", [P, T]), sb("x1", [P, T])]
        dt = [sb("d0", [P, T]), sb("d1", [P, T])]
        ones = sb("ones", [P, P])
        sn = sb("sn", [P, RB]); snp = sb("snp", [P, RB]); red1 = sb("red1", [P, 2])
        tot1 = sb("tot1", [P, 2]); rc = sb("rc", [P, 1]); mean = sb("mean", [P, 1])
        dev = sb("dev", [P, RB]); dv2 = sb("dv2", [P, RB]); dv2p = sb("dv2p", [P, RB])
        red2 = sb("red2", [P, 1]); vs = sb("vs", [P, 1]); cm1 = sb("cm1", [P, 1])
        rcm1 = sb("rcm1", [P, 1]); var = sb("var", [P, 1]); std = sb("std", [P, 1])
        stde = sb("stde", [P, 1]); rstd = sb("rstd", [P, 1]); ms = sb("ms", [P, RB])
        g = sb("g", [P, RB]); gadd = sb("gadd", [P, RB])
        b_hpi = sb("b_hpi", [P, 1]); b_hpe = sb("b_hpe", [P, 1])
        cg = sb("cg", [P, RB]); sg = sb("sg", [P, RB])
        x2 = sb("x2", [P, RB]); sq = sb("sq", [P, RB])
        t1 = sb("t1", [P, RB]); t2 = sb("t2", [P, RB]); tt = sb("tt", [P, RB])
        u = sb("u", [P, RB]); w = sb("w", [P, RB])
        ca = sb("ca", [P, RB]); cb = sb("cb", [P, RB])
        chi = sb("chi", [P, RB], u32); u2 = sb("u2", [P, RB])
        cc = sb("cc", [P, RB]); cd = sb("cd", [P, RB])
        clo = sb("clo", [P, RB], u32)
        negc = sb("negc", [P, RB]); posc = sb("posc", [P, RB])
        vhat = sb("vhat", [P, RB]); vfin = sb("vfin", [P, RB])
        dvx = sb("dvx", [P, RB]); d64 = sb("d64", [P, RB])
        delta = sb("delta", [P, RB]); locadj = sb("locadj", [P, CT * RB])
        ps1 = psb("ps1", [P, 2]); ps2 = psb("ps2", [P, 1])

        nrm_t = sd[:, 0 * RB : 1 * RB]
        pos_t = sd[:, 1 * RB : 2 * RB]
        m_t = sd[:, 2 * RB : 3 * RB]
        loc_t = sd[:, 3 * RB : 4 * RB]
        xvv = sd[:, 4 * RB : 5 * RB]

        dA = sem("sm_dma")   # sidecar+ramp loads      (+16 each)
        sL = sem("loads")    # stream tile loads       (+16 each)
        sS = sem("stores")   # stream tile stores      (+16 each)
        sC = sem("compute")  # per-tile fused op done  (+1 each)
        hDP = sem("dve2pe")
        hPD = sem("pe2dve")
        hDA = sem("dve2act")
        hAD = sem("act2dve")

        with nc.Block() as block:

            @block.gpsimd
            def _(gp):
                gp.dma_start(out=sd, in_=sdc[:]).then_inc(dA, 16)
                gp.dma_start(out=ramp, in_=rmp[:]).then_inc(dA, 16)
                gp.dma_start(out=xt[0], in_=tileslice(lg2d, 0)).then_inc(sL, 16)
                gp.dma_start(out=xt[1], in_=tileslice(lg2d, 1)).then_inc(sL, 16)
                for k in range(NTILES):
                    gp.wait_ge(sC, k + 1)
                    gp.dma_start(out=tileslice(out2d, k), in_=dt[k % 2]).then_inc(
                        sS, 16
                    )
                    if k + 2 < NTILES:
                        gp.dma_start(
                            out=xt[k % 2], in_=tileslice(lg2d, k + 2)
                        ).then_inc(sL, 16)
                gp.wait_ge(sS, 16 * NTILES)

            @block.vector
            def _(v):
                v.memset(ones, 1.0)
                v.memset(b_hpi, PI / 2)
                v.memset(b_hpe, PI / 2 + EPS)
                v.memset(negc, -COS_EPS)
                v.memset(posc, COS_EPS)
                v.wait_ge(dA, 32)
                # NOTE: raw-bass DVE ops pipeline; dependent back-to-back ops
                # need explicit drains, and cross-engine sem incs must ride a
                # drain so the write-back is visible to the consumer.
                v.tensor_scalar(sn, nrm_t, 1e-3, 100.0, Alu.max, Alu.min)
                v.drain()
                v.tensor_tensor(snp, sn, pos_t, Alu.mult)
                v.drain()
                v.tensor_reduce(red1[:, 0:1], snp, axis=AxX, op=Alu.add)
                v.drain()
                v.tensor_reduce(red1[:, 1:2], pos_t, axis=AxX, op=Alu.add)
                v.drain().then_inc(hDP, 1)
                v.wait_ge(hPD, 1)
                v.tensor_copy(tot1, ps1)
                v.drain()
                v.reciprocal(rc, tot1[:, 1:2])
                v.drain()
                v.tensor_tensor(mean, tot1[:, 0:1], rc, Alu.mult)
                v.drain()
                v.tensor_scalar(dev, sn, mean, None, Alu.subtract)
                v.drain()
                v.tensor_tensor(dv2, dev, dev, Alu.mult)
                v.drain()
                v.tensor_tensor(dv2p, dv2, pos_t, Alu.mult)
                v.drain()
                v.tensor_reduce(red2, dv2p, axis=AxX, op=Alu.add)
                v.drain().then_inc(hDP, 1)
                v.wait_ge(hPD, 2)
                v.tensor_copy(vs, ps2)
                v.drain()
                v.tensor_scalar_add(cm1, tot1[:, 1:2], -1.0)
                v.drain()
                v.reciprocal(rcm1, cm1)
                v.drain()
                v.tensor_tensor(var, vs, rcm1, Alu.mult)
                v.drain().then_inc(hDA, 1)
                v.wait_ge(hAD, 1)
                v.tensor_scalar_add(stde, std, EPS)
                v.drain()
                v.reciprocal(rstd, stde)
                v.drain()
                v.tensor_scalar(ms, dev, rstd, None, Alu.mult)
                v.drain()
                v.tensor_scalar_mul(g, ms, -M_C)
                v.drain()
                v.tensor_scalar(gadd, ms, M_C, M_C, Alu.mult, Alu.add)
                v.drain().then_inc(hDA, 1)
                v.wait_ge(hAD, 2)
                v.tensor_tensor(t1, xvv, cg, Alu.mult)
                v.drain()
                v.tensor_tensor(t2, sq, sg, Alu.mult)
                v.drain()
                v.tensor_tensor(tt, t1, t2, Alu.subtract)
                v.drain()
                # clip-high: theta+g > pi-eps <=> (g >= -eps) & (xv+cos(g+eps) < 0)
                v.tensor_tensor(w, xvv, u, Alu.add)
                v.drain()
                v.tensor_scalar(ca, g, -EPS, None, Alu.is_ge)
                v.drain()
                v.tensor_scalar(cb, w, 0.0, None, Alu.is_lt)
                v.drain()
                v.tensor_tensor(chi, ca, cb, Alu.mult)
                v.drain()
                # clip-low: theta+g < eps <=> (g <= eps) & (xv > cos(eps-g))
                v.tensor_tensor(cc, xvv, u2, Alu.is_gt)
                v.drain()
                v.tensor_scalar(cd, g, EPS, None, Alu.is_le)
                v.drain()
                v.tensor_tensor(clo, cc, cd, Alu.mult)
                v.drain()
                v.tensor_copy(vhat, tt)
                v.drain()
                v.copy_predicated(vhat, chi, negc)
                v.drain()
                v.copy_predicated(vhat, clo, posc)
                v.drain()
                v.tensor_tensor(vfin, vhat, gadd, Alu.subtract)
                v.drain()
                v.tensor_tensor(dvx, vfin, xvv, Alu.subtract)
                v.drain()
                v.tensor_scalar_mul(d64, dvx, S)
                v.drain()
                v.tensor_tensor(delta, d64, m_t, Alu.mult)
                v.drain()
                for ct in range(CT):
                    v.tensor_scalar_add(
                        locadj[:, ct * RB : (ct + 1) * RB], loc_t, -float(ct * T)
                    )
                v.drain()
                for k in range(NTILES):
                    rb, ct = divmod(k, CT)
                    if k >= 2:
                        v.wait_ge(sS, 16 * (k - 1))
                    v.tensor_scalar(
                        dt[k % 2],
                        ramp,
                        locadj[:, ct * RB + rb : ct * RB + rb + 1],
                        delta[:, rb : rb + 1],
                        Alu.is_equal,
                        Alu.mult,
                    )
                    v.drain()
                    v.wait_ge(sL, 16 * (k + 1))
                    v.scalar_tensor_tensor(
                        dt[k % 2], xt[k % 2], S, dt[k % 2], Alu.mult, Alu.add
                    )
                    v.drain().then_inc(sC, 1)

            @block.scalar
            def _(sc):
                sc.wait_ge(hDA, 1)
                sc.activation(std, var, Act.Sqrt)
                sc.drain().then_inc(hAD, 1)
                sc.wait_ge(hDA, 2)
                sc.activation(cg, g, Act.Sin, bias=b_hpi)
                sc.activation(sg, g, Act.Sin)
                sc.activation(x2, xvv, Act.Square)
                sc.drain()
                sc.activation(sq, x2, Act.Sqrt, scale=-1.0, bias=1.0)
                sc.activation(u, g, Act.Sin, bias=b_hpe)
                sc.activation(u2, g, Act.Sin, scale=-1.0, bias=b_hpe)
                sc.drain().then_inc(hAD, 1)

            @block.tensor
            def _(te):
                te.wait_ge(hDP, 1)
                te.matmul(ps1, lhsT=ones, rhs=red1, start=True, stop=True)
                te.drain().then_inc(hPD, 1)
                te.wait_ge(hDP, 2)
                te.matmul(ps2, lhsT=ones, rhs=red2, start=True, stop=True)
                te.drain().then_inc(hPD, 1)

    return nc


def _get_program():
    if "nc" not in _CACHED:
        _CACHED["nc"] = _build_program()
    return _CACHED["nc"]


def _prep_inputs(logits, norms, labels):
    """Shard across 8 cores; build per-core index/mask sidecar tensors."""
    labels = np.asarray(labels).astype(np.int64)
    logits = np.asarray(logits, dtype=np.float32)
    norms = np.asarray(norms, dtype=np.float32)

    rows = np.arange(B, dtype=np.int64)
    posf = (labels >= 0).astype(np.float32)

    def fold(a):
        # [B] -> [P, RB] with element (p, rb) = row rb*P + p
        return np.ascontiguousarray(a.reshape(RB, P).T)

    norms_f = fold(norms[:, 0])
    posf_f = fold(posf)

    in_maps = []
    xv = logits[rows, np.clip(labels, 0, C - 1)].astype(np.float32)
    xv_f = fold(xv)
    ramp = np.ascontiguousarray(
        np.broadcast_to(np.arange(T, dtype=np.float32), (P, T))
    )
    for m in range(NCORES):
        c0 = m * CSH
        loc = labels - c0
        inr = (labels >= 0) & (loc >= 0) & (loc < CSH)
        locf = np.where(inr, loc, -1).astype(np.float32)
        shard = np.ascontiguousarray(logits[:, c0 : c0 + CSH]).reshape(-1)
        sidecar = np.concatenate(
            [
                norms_f,
                posf_f,
                fold(inr.astype(np.float32)),
                fold(locf),
                xv_f,
            ],
            axis=1,
        )
        in_maps.append(
            {
                "logits": shard,
                "sidecar": np.ascontiguousarray(sidecar),
                "ramp": ramp,
            }
        )
    return in_maps


def kernel(logits, norms, labels, _trace=False, _trace_kwargs=None):
    from concourse import bass_utils

    nc = _get_program()
    in_maps = _prep_inputs(logits, norms, labels)
    res = bass_utils.run_bass_kernel_spmd(
        nc,
        in_maps,
        core_ids=list(range(NCORES)),
        trace=_trace,
        **(_trace_kwargs or {}),
    )
    _CACHED["last_result"] = res
    shards = [res.results[i]["out"].reshape(B, CSH) for i in range(NCORES)]
    return np.concatenate(shards, axis=1)
